# revision 22
# baseline (speedup 1.0000x reference)
"""TRN2 Bass kernel for nn_BimodalAttention.

Reference computation (B=16, T=2048, D1=D2=1024, U=1024):
    f1 = X1 @ W1 + b1 ; f2 = X2 @ W2 + b2
    H  = tanh(concat(f1, f2) @ W + b)            # [B,T,U]
    s  = H @ c ; a = softmax(s, axis=T)          # [B,T,1]
    out[b] = sum_t a[b,t] * H[b,t]               # [B,U]

Device strategy (data-parallel over batch, 2 batches per core, 8 cores):
  * Host folds the linear chain: M1 = W1 @ W[:U], M2 = W2 @ W[U:], so the
    device computes H = tanh(Xcat @ M + beff) with M = [M1; M2] — half the
    matmul FLOPs of the literal graph.
  * Host pre-transposes/tiles Xcat to [B, K/128, 128, T] so every lhsT tile
    DMA is contiguous, and replicates the context vector across the 128
    partitions so scores are row-local DVE work.
  * Main matmuls run as float32r (full PE rate, ~1.7e-4 matmul rel-err).
    DRAM inputs are declared float32r directly — HW does its own rounding,
    so plain HWDGE loads work and no casting DMAs are needed.
  * Softmax over T: no max-subtraction (scores are ~N(0,10) by
    construction; exp overflows only past 88) — a clamp at 60 guards
    against inf.  exp is therefore per-element, so the weighted time-sum
    (PE matmuls with the unnormalized exp weights stationary) streams
    through phase A chunk by chunk; only 1/Z normalization waits for the
    end.  Z (cross-partition sum) comes from a tiny fp32 matmul with a
    ones vector.
"""
import numpy as np

import concourse.bacc as bacc
import concourse.mybir as mybir
from concourse.bass_utils import run_bass_kernel_spmd
from concourse.tile import TileContext

F32 = mybir.dt.float32
F32R = mybir.dt.float32r

N_CORES = 8
B, T, D, UNITS = 16, 2048, 1024, 1024
KD = 2 * D          # folded contraction dim (seq1 ++ seq2)
BPC = B // N_CORES  # batches per core

_NC_CACHE = {}


def build_nc(bpc=BPC, t=T, kd=KD, units=UNITS, has_bias=False, tchunk=512):
    """Build the per-core Bass module (same program on all cores)."""
    nc = bacc.Bacc(None, target_bir_lowering=False)

    nk = kd // 128              # k-blocks in contraction
    nt = t // 128               # t-blocks
    nuh = (units + 511) // 512  # 512-wide u column groups
    uh_w = units // nuh
    ntc = t // tchunk           # streamed X chunks per batch
    tpc = tchunk // 128         # t-blocks per chunk

    xt = nc.declare_dram_parameter("xt", [bpc, nk, 128, t], F32R, isOutput=False)
    mw = nc.declare_dram_parameter("mw", [nk, 128, units], F32R, isOutput=False)
    crep = nc.declare_dram_parameter("crep", [128, units], F32, isOutput=False)
    brep = nc.declare_dram_parameter("brep", [128, units], F32, isOutput=False)
    out = nc.declare_dram_parameter("out", [bpc, units], F32, isOutput=True)

    with TileContext(nc) as tc:
        with (
            tc.tile_pool(name="wpool", bufs=1) as wpool,
            tc.tile_pool(name="xpool", bufs=2) as xpool,
            tc.tile_pool(name="hpool", bufs=tpc + 2) as hpool,
            tc.tile_pool(name="spool", bufs=2) as spool,
            tc.tile_pool(name="sppool", bufs=6) as sppool,
            tc.tile_pool(name="scratch", bufs=2) as scratch,
            tc.tile_pool(name="mainps", bufs=4, space="PSUM") as mainps,
            tc.tile_pool(name="outps", bufs=1, space="PSUM") as outps,
            tc.tile_pool(name="zps", bufs=2, space="PSUM") as zps,
        ):
            # ---- resident small tensors -------------------------------
            # The first psum group consumes k-blocks in order, so the
            # critical path to the first matmul is only the first k-quarter
            # of the uh=0 weight half plus the first k-quarter of X chunk 0.
            # Interleave quarter-loads of both so PE starts after ~2MB.
            mwt = wpool.tile([128, nk * units], F32R, name="mwt")
            mwt4 = mwt.rearrange("p (k h u) -> p k h u", k=nk, h=nuh)
            kq = max(1, nk // 4)
            mw_r = mw.rearrange("k p u -> p k u")
            crep_s = wpool.tile([128, units], F32, name="crep_s")
            ones_s = wpool.tile([128, 1], F32, name="ones_s")
            nc.vector.memset(ones_s[:, :], 1.0)
            if has_bias:
                brep_s = wpool.tile([128, units], F32, name="brep_s")
                nc.sync.dma_start(out=brep_s[:, :], in_=brep[:, :])

            first_deferred = True
            for b in range(bpc):
                s_all = spool.tile([128, nt], F32, tag="s_all", name="s_all")
                s_c = spool.tile([128, nt], F32, tag="s_c", name="s_c")
                e_f32 = spool.tile([128, nt], F32, tag="e_f32", name="e_f32")
                e_all = spool.tile([128, nt], F32R, tag="e_all", name="e_all")
                o_ps = outps.tile([1, units], F32, tag="o_ps", name="o_ps")
                wsum_pending = []

                if b == 0:
                    # PE warm-up: dummy matmuls into o_ps (the real t0=0
                    # weighted-sum matmul re-clears it with start=True).
                    # Gets HAM to K=8/8 while the first loads stream in.
                    warm = wpool.tile([128, uh_w], F32R, name="warm")
                    nc.sync.dma_start(out=warm[:, :], in_=mw[0, :, 0:uh_w])
                    for _ in range(16):
                        nc.tensor.matmul(
                            out=o_ps[0:1, 0:uh_w],
                            lhsT=warm[:, 0:1], rhs=warm[:, 0:uh_w],
                            start=True, stop=True,
                        )

                for tcix in range(ntc):
                    first_chunk = first_deferred
                    a_t = xpool.tile([128, nk * tchunk], F32R, tag="a_t",
                                     name="a_t")
                    a_t3 = a_t.rearrange("p (k w) -> p k w", k=nk)
                    x_src = xt[b].rearrange("k p w -> p k w")[
                        :, :, tcix * tchunk:(tcix + 1) * tchunk]
                    if first_chunk:
                        # interleaved k-quarter loads of mw[uh0] and chunk 0
                        for q in range(0, nk, kq):
                            nc.sync.dma_start(
                                out=mwt4[:, q:q + kq, 0, :],
                                in_=mw_r[:, q:q + kq, 0:uh_w],
                            )
                            nc.sync.dma_start(
                                out=a_t3[:, q:q + kq, :],
                                in_=x_src[:, q:q + kq, :],
                            )
                        for uh in range(1, nuh):
                            for q in range(0, nk, kq):
                                nc.sync.dma_start(
                                    out=mwt4[:, q:q + kq, uh, :],
                                    in_=mw_r[:, q:q + kq,
                                             uh * uh_w:(uh + 1) * uh_w],
                                )
                        nc.sync.dma_start(out=crep_s[:, :], in_=crep[:, :])
                        first_deferred = False
                    else:
                        nc.sync.dma_start(out=a_t3, in_=x_src)

                    # chunk 0 runs uh-outer so the uh=0 groups (whose weights
                    # arrive first) fully precede the uh=1 groups.
                    if first_chunk:
                        pair_order = [(i, uh) for uh in range(nuh)
                                      for i in range(tpc)]
                    else:
                        pair_order = [(i, uh) for i in range(tpc)
                                      for uh in range(nuh)]
                    h_tmps = {}
                    h_ts = {}
                    sp_tiles = {}
                    done_count = {}
                    for i, uh in pair_order:
                        t0 = tcix * tpc + i
                        if i not in h_tmps:
                            h_tmps[i] = scratch.tile(
                                [128, units], F32, tag="h_tmp",
                                name="h_tmp", bufs=tpc + 1)
                            h_ts[i] = hpool.tile([128, units], F32R, tag="H",
                                                 name="h_t")
                            done_count[i] = 0
                        h_tmp, h_t = h_tmps[i], h_ts[i]
                        ps = mainps.tile([128, uh_w], F32, tag="ps", name="ps")
                        for k in range(nk):
                            nc.tensor.matmul(
                                out=ps[:, :],
                                lhsT=a_t[:, k * tchunk + i * 128:
                                         k * tchunk + (i + 1) * 128],
                                rhs=mwt[:, k * units + uh * uh_w:
                                        k * units + (uh + 1) * uh_w],
                                start=(k == 0),
                                stop=(k == nk - 1),
                            )
                        if has_bias:
                            nc.vector.tensor_tensor(
                                out=ps[:, :], in0=ps[:, :],
                                in1=brep_s[:, uh * uh_w:(uh + 1) * uh_w],
                                op=mybir.AluOpType.add,
                            )
                        nc.scalar.activation(
                            out=h_tmp[:, uh * uh_w:(uh + 1) * uh_w],
                            in_=ps[:, :],
                            func=mybir.ActivationFunctionType.Tanh,
                        )
                        # partial scores for this u-half right away, so only
                        # the last half's reduction trails the final matmul
                        uhs = slice(uh * uh_w, (uh + 1) * uh_w)
                        junk = scratch.tile([128, uh_w], F32, tag="junk",
                                            name="junk", bufs=3)
                        if i not in sp_tiles:
                            sp_tiles[i] = sppool.tile([128, nuh], F32,
                                                      tag="sp", name="sp")
                        sp = sp_tiles[i]
                        nc.vector.tensor_mul(junk[:, :], h_tmp[:, uhs],
                                             crep_s[:, uhs])
                        nc.vector.reduce_sum(
                            out=sp[:, uh:uh + 1], in_=junk[:, :],
                            axis=mybir.AxisListType.X,
                        )
                        # f32r copy of this half for the weighted-sum
                        # matmul (sync DMA: overlaps the DVE scores chain)
                        nc.sync.dma_start(out=h_t[:, uhs],
                                          in_=h_tmp.bitcast(F32R)[:, uhs])
                        done_count[i] += 1
                        if done_count[i] < nuh:
                            continue
                        # ---- tile epilogue: all u-halves of t0 done ----
                        if nuh > 1:
                            nc.vector.reduce_sum(
                                out=s_all[:, t0:t0 + 1], in_=sp[:, :],
                                axis=mybir.AxisListType.X,
                            )
                        else:
                            nc.vector.tensor_copy(s_all[:, t0:t0 + 1],
                                                  sp[:, :])
                        # e = exp(clamp(s)) for this tile, f32r bit-copy
                        nc.vector.tensor_scalar_min(
                            s_c[:, t0:t0 + 1], s_all[:, t0:t0 + 1], 60.0)
                        nc.scalar.activation(
                            out=e_f32[:, t0:t0 + 1], in_=s_c[:, t0:t0 + 1],
                            func=mybir.ActivationFunctionType.Exp,
                        )
                        nc.vector.tensor_copy(e_all[:, t0:t0 + 1],
                                              e_f32[:, t0:t0 + 1])
                        # queue this tile's weighted-sum matmuls; emit the
                        # previous tile's now (one-tile pipeline slack so PE
                        # never waits on the scores->exp chain)
                        wsum_pending.append((t0, h_t))
                        if len(wsum_pending) > 1:
                            pt0, ph = wsum_pending.pop(0)
                            for wuh in range(nuh):
                                nc.tensor.matmul(
                                    out=o_ps[0:1, wuh * uh_w:(wuh + 1) * uh_w],
                                    lhsT=e_all[:, pt0:pt0 + 1],
                                    rhs=ph[:, wuh * uh_w:(wuh + 1) * uh_w],
                                    start=(pt0 == 0),
                                    stop=(pt0 == nt - 1),
                                )

                for pt0, ph in wsum_pending:
                    for wuh in range(nuh):
                        nc.tensor.matmul(
                            out=o_ps[0:1, wuh * uh_w:(wuh + 1) * uh_w],
                            lhsT=e_all[:, pt0:pt0 + 1],
                            rhs=ph[:, wuh * uh_w:(wuh + 1) * uh_w],
                            start=(pt0 == 0),
                            stop=(pt0 == nt - 1),
                        )
                wsum_pending = []

                # ---- normalization: o = o' / Z -----------------------
                esum = spool.tile([128, 1], F32, tag="esum", name="esum")
                nc.vector.reduce_sum(out=esum[:, :], in_=e_f32[:, :],
                                     axis=mybir.AxisListType.X)
                z_ps = zps.tile([1, 1], F32, tag="z_ps", name="z_ps")
                nc.tensor.matmul(out=z_ps[:, :], lhsT=ones_s[:, :],
                                 rhs=esum[:, :], start=True, stop=True)
                rz = spool.tile([1, 1], F32, tag="rz", name="rz")
                nc.vector.reciprocal(rz[:, :], z_ps[:, :])
                o_sb = scratch.tile([1, units], F32, tag="o_sb", name="o_sb")
                nc.vector.tensor_scalar_mul(o_sb[:, :], o_ps[:, :],
                                            rz[0:1, 0:1])
                nc.sync.dma_start(out=out[b:b + 1, :], in_=o_sb[:, :])

    nc.finalize()
    return nc


def _prep_inputs(sequences1, sequences2, W1_kernel, W1_bias, W2_kernel,
                 W2_bias, W_kernel, W_bias, context_vector):
    """Host-side folding + layout. Returns (per-core in_maps, has_bias)."""
    U = UNITS
    W = np.asarray(W_kernel, np.float32)
    M1 = np.asarray(W1_kernel, np.float32) @ W[:U]
    M2 = np.asarray(W2_kernel, np.float32) @ W[U:]
    beff = (np.asarray(W1_bias, np.float32) @ W[:U]
            + np.asarray(W2_bias, np.float32) @ W[U:]
            + np.asarray(W_bias, np.float32))
    has_bias = bool(np.any(beff != 0.0))

    M = np.concatenate([M1, M2], axis=0)                   # [KD, U]
    mw = np.ascontiguousarray(M.reshape(KD // 128, 128, U), np.float32)
    c = np.asarray(context_vector, np.float32).reshape(U)
    crep = np.ascontiguousarray(np.broadcast_to(c, (128, U)), np.float32)
    brep = np.ascontiguousarray(np.broadcast_to(beff, (128, U)), np.float32)

    x1 = np.asarray(sequences1, np.float32)
    x2 = np.asarray(sequences2, np.float32)
    in_maps = []
    for core in range(N_CORES):
        bs = slice(core * BPC, (core + 1) * BPC)
        xcat = np.concatenate([x1[bs], x2[bs]], axis=2)    # [BPC, T, KD]
        # -> [BPC, KD/128, 128, T]: xt[b, k, p, t] = xcat[b, t, 128k + p]
        xtc = np.ascontiguousarray(
            xcat.transpose(0, 2, 1).reshape(BPC, KD // 128, 128, T)
        )
        in_maps.append({"xt": xtc, "mw": mw, "crep": crep, "brep": brep})
    return in_maps, has_bias


def kernel(sequences1, sequences2, W1_kernel, W1_bias, W2_kernel, W2_bias,
           W_kernel, W_bias, context_vector):
    in_maps, has_bias = _prep_inputs(
        sequences1, sequences2, W1_kernel, W1_bias, W2_kernel, W2_bias,
        W_kernel, W_bias, context_vector)
    key = ("full", has_bias)
    if key not in _NC_CACHE:
        _NC_CACHE[key] = build_nc(has_bias=has_bias)
    nc = _NC_CACHE[key]
    res = run_bass_kernel_spmd(nc, in_maps, list(range(N_CORES)))
    return np.concatenate([r["out"] for r in res.results], axis=0)


# revision 23
# speedup vs baseline: 1.0310x; 1.0310x over previous
"""TRN2 Bass kernel for nn_BimodalAttention.

Reference computation (B=16, T=2048, D1=D2=1024, U=1024):
    f1 = X1 @ W1 + b1 ; f2 = X2 @ W2 + b2
    H  = tanh(concat(f1, f2) @ W + b)            # [B,T,U]
    s  = H @ c ; a = softmax(s, axis=T)          # [B,T,1]
    out[b] = sum_t a[b,t] * H[b,t]               # [B,U]

Device strategy (data-parallel over batch, 2 batches per core, 8 cores):
  * Host folds the linear chain: M1 = W1 @ W[:U], M2 = W2 @ W[U:], so the
    device computes H = tanh(Xcat @ M + beff) with M = [M1; M2] — half the
    matmul FLOPs of the literal graph.
  * Host pre-transposes/tiles Xcat to [B, K/128, 128, T] so every lhsT tile
    DMA is contiguous, and replicates the context vector across the 128
    partitions so scores are row-local DVE work.
  * Main matmuls run as float32r (full PE rate, ~1.7e-4 matmul rel-err).
    DRAM inputs are declared float32r directly — HW does its own rounding,
    so plain HWDGE loads work and no casting DMAs are needed.
  * Softmax over T: no max-subtraction (scores are ~N(0,10) by
    construction; exp overflows only past 88) — a clamp at 60 guards
    against inf.  exp is therefore per-element, so the weighted time-sum
    (PE matmuls with the unnormalized exp weights stationary) streams
    through phase A chunk by chunk; only 1/Z normalization waits for the
    end.  Z (cross-partition sum) comes from a tiny fp32 matmul with a
    ones vector.
"""
import numpy as np

import concourse.bacc as bacc
import concourse.mybir as mybir
from concourse.bass_utils import run_bass_kernel_spmd
from concourse.tile import TileContext

F32 = mybir.dt.float32
F32R = mybir.dt.float32r

N_CORES = 8
B, T, D, UNITS = 16, 2048, 1024, 1024
KD = 2 * D          # folded contraction dim (seq1 ++ seq2)
BPC = B // N_CORES  # batches per core

_NC_CACHE = {}


def build_nc(bpc=BPC, t=T, kd=KD, units=UNITS, has_bias=False, tchunk=512):
    """Build the per-core Bass module (same program on all cores)."""
    nc = bacc.Bacc(None, target_bir_lowering=False)

    nk = kd // 128              # k-blocks in contraction
    nt = t // 128               # t-blocks
    nuh = (units + 511) // 512  # 512-wide u column groups
    uh_w = units // nuh
    ntc = t // tchunk           # streamed X chunks per batch
    tpc = tchunk // 128         # t-blocks per chunk

    xt = nc.declare_dram_parameter("xt", [bpc, nk, 128, t], F32R, isOutput=False)
    mw = nc.declare_dram_parameter("mw", [nk, 128, units], F32R, isOutput=False)
    crep = nc.declare_dram_parameter("crep", [128, units], F32, isOutput=False)
    brep = nc.declare_dram_parameter("brep", [128, units], F32, isOutput=False)
    out = nc.declare_dram_parameter("out", [bpc, units], F32, isOutput=True)

    with TileContext(nc) as tc:
        with (
            tc.tile_pool(name="wpool", bufs=1) as wpool,
            tc.tile_pool(name="xpool", bufs=2) as xpool,
            tc.tile_pool(name="hpool", bufs=tpc + 2) as hpool,
            tc.tile_pool(name="spool", bufs=2) as spool,
            tc.tile_pool(name="sppool", bufs=6) as sppool,
            tc.tile_pool(name="scratch", bufs=2) as scratch,
            tc.tile_pool(name="mainps", bufs=4, space="PSUM") as mainps,
            tc.tile_pool(name="outps", bufs=1, space="PSUM") as outps,
            tc.tile_pool(name="zps", bufs=2, space="PSUM") as zps,
        ):
            # ---- resident small tensors -------------------------------
            # The first psum group consumes k-blocks in order, so the
            # critical path to the first matmul is only the first k-quarter
            # of the uh=0 weight half plus the first k-quarter of X chunk 0.
            # Interleave quarter-loads of both so PE starts after ~2MB.
            mwt = wpool.tile([128, nk * units], F32R, name="mwt")
            mwt4 = mwt.rearrange("p (k h u) -> p k h u", k=nk, h=nuh)
            kq = max(1, nk // 4)
            mw_r = mw.rearrange("k p u -> p k u")
            crep_s = wpool.tile([128, units], F32, name="crep_s")
            ones_s = wpool.tile([128, 1], F32, name="ones_s")
            nc.vector.memset(ones_s[:, :], 1.0)
            if has_bias:
                brep_s = wpool.tile([128, units], F32, name="brep_s")
                nc.sync.dma_start(out=brep_s[:, :], in_=brep[:, :])

            first_deferred = True
            for b in range(bpc):
                s_all = spool.tile([128, nt], F32, tag="s_all", name="s_all")
                s_c = spool.tile([128, nt], F32, tag="s_c", name="s_c")
                e_f32 = spool.tile([128, nt], F32, tag="e_f32", name="e_f32")
                e_all = spool.tile([128, nt], F32R, tag="e_all", name="e_all")
                o_ps = outps.tile([1, units], F32, tag="o_ps", name="o_ps")
                wsum_pending = []

                if b == 0:
                    # PE warm-up: dummy matmuls into o_ps (the real t0=0
                    # weighted-sum matmul re-clears it with start=True).
                    # Gets HAM to K=8/8 while the first loads stream in.
                    warm = wpool.tile([128, uh_w], F32R, name="warm")
                    nc.sync.dma_start(out=warm[:, :], in_=mw[0, :, 0:uh_w])
                    for _ in range(16):
                        nc.tensor.matmul(
                            out=o_ps[0:1, 0:uh_w],
                            lhsT=warm[:, 0:1], rhs=warm[:, 0:uh_w],
                            start=True, stop=True,
                        )

                for tcix in range(ntc):
                    first_chunk = first_deferred
                    a_t = xpool.tile([128, nk * tchunk], F32R, tag="a_t",
                                     name="a_t")
                    a_t3 = a_t.rearrange("p (k w) -> p k w", k=nk)
                    x_src = xt[b].rearrange("k p w -> p k w")[
                        :, :, tcix * tchunk:(tcix + 1) * tchunk]
                    if first_chunk:
                        # interleaved k-quarter loads of mw[uh0] and chunk 0
                        for q in range(0, nk, kq):
                            nc.sync.dma_start(
                                out=mwt4[:, q:q + kq, 0, :],
                                in_=mw_r[:, q:q + kq, 0:uh_w],
                            )
                            nc.sync.dma_start(
                                out=a_t3[:, q:q + kq, :],
                                in_=x_src[:, q:q + kq, :],
                            )
                        for uh in range(1, nuh):
                            for q in range(0, nk, kq):
                                nc.sync.dma_start(
                                    out=mwt4[:, q:q + kq, uh, :],
                                    in_=mw_r[:, q:q + kq,
                                             uh * uh_w:(uh + 1) * uh_w],
                                )
                        nc.sync.dma_start(out=crep_s[:, :], in_=crep[:, :])
                        first_deferred = False
                    else:
                        nc.sync.dma_start(out=a_t3, in_=x_src)

                    # chunk 0 runs uh-outer so the uh=0 groups (whose weights
                    # arrive first) fully precede the uh=1 groups.
                    if first_chunk:
                        pair_order = [(i, uh) for uh in range(nuh)
                                      for i in range(tpc)]
                    else:
                        pair_order = [(i, uh) for i in range(tpc)
                                      for uh in range(nuh)]
                    h_tmps = {}
                    h_ts = {}
                    sp_tiles = {}
                    done_count = {}
                    for i, uh in pair_order:
                        t0 = tcix * tpc + i
                        if i not in h_tmps:
                            h_tmps[i] = scratch.tile(
                                [128, units], F32, tag="h_tmp",
                                name="h_tmp", bufs=tpc + 1)
                            h_ts[i] = hpool.tile([128, units], F32R, tag="H",
                                                 name="h_t")
                            done_count[i] = 0
                        h_tmp, h_t = h_tmps[i], h_ts[i]
                        ps = mainps.tile([128, uh_w], F32, tag="ps", name="ps")
                        for k in range(nk):
                            nc.tensor.matmul(
                                out=ps[:, :],
                                lhsT=a_t[:, k * tchunk + i * 128:
                                         k * tchunk + (i + 1) * 128],
                                rhs=mwt[:, k * units + uh * uh_w:
                                        k * units + (uh + 1) * uh_w],
                                start=(k == 0),
                                stop=(k == nk - 1),
                            )
                        if has_bias:
                            nc.vector.tensor_tensor(
                                out=ps[:, :], in0=ps[:, :],
                                in1=brep_s[:, uh * uh_w:(uh + 1) * uh_w],
                                op=mybir.AluOpType.add,
                            )
                        nc.scalar.activation(
                            out=h_tmp[:, uh * uh_w:(uh + 1) * uh_w],
                            in_=ps[:, :],
                            func=mybir.ActivationFunctionType.Tanh,
                        )
                        # partial scores for this u-half right away, so only
                        # the last half's reduction trails the final matmul
                        uhs = slice(uh * uh_w, (uh + 1) * uh_w)
                        junk = scratch.tile([128, uh_w], F32, tag="junk",
                                            name="junk", bufs=3)
                        if i not in sp_tiles:
                            sp_tiles[i] = sppool.tile([128, nuh], F32,
                                                      tag="sp", name="sp")
                        sp = sp_tiles[i]
                        nc.vector.tensor_mul(junk[:, :], h_tmp[:, uhs],
                                             crep_s[:, uhs])
                        nc.vector.reduce_sum(
                            out=sp[:, uh:uh + 1], in_=junk[:, :],
                            axis=mybir.AxisListType.X,
                        )
                        # f32r copy of this half for the weighted-sum matmul
                        nc.vector.tensor_copy(h_t[:, uhs], h_tmp[:, uhs])
                        done_count[i] += 1
                        if done_count[i] < nuh:
                            continue
                        # ---- tile epilogue: all u-halves of t0 done ----
                        if nuh > 1:
                            nc.vector.reduce_sum(
                                out=s_all[:, t0:t0 + 1], in_=sp[:, :],
                                axis=mybir.AxisListType.X,
                            )
                        else:
                            nc.vector.tensor_copy(s_all[:, t0:t0 + 1],
                                                  sp[:, :])
                        # e = exp(clamp(s)) for this tile, f32r bit-copy
                        nc.vector.tensor_scalar_min(
                            s_c[:, t0:t0 + 1], s_all[:, t0:t0 + 1], 60.0)
                        nc.scalar.activation(
                            out=e_f32[:, t0:t0 + 1], in_=s_c[:, t0:t0 + 1],
                            func=mybir.ActivationFunctionType.Exp,
                        )
                        nc.vector.tensor_copy(e_all[:, t0:t0 + 1],
                                              e_f32[:, t0:t0 + 1])
                        # queue this tile's weighted-sum matmuls; emit the
                        # previous tile's now (one-tile pipeline slack so PE
                        # never waits on the scores->exp chain)
                        wsum_pending.append((t0, h_t))
                        if len(wsum_pending) > 1:
                            pt0, ph = wsum_pending.pop(0)
                            for wuh in range(nuh):
                                nc.tensor.matmul(
                                    out=o_ps[0:1, wuh * uh_w:(wuh + 1) * uh_w],
                                    lhsT=e_all[:, pt0:pt0 + 1],
                                    rhs=ph[:, wuh * uh_w:(wuh + 1) * uh_w],
                                    start=(pt0 == 0),
                                    stop=(pt0 == nt - 1),
                                )

                for pt0, ph in wsum_pending:
                    for wuh in range(nuh):
                        nc.tensor.matmul(
                            out=o_ps[0:1, wuh * uh_w:(wuh + 1) * uh_w],
                            lhsT=e_all[:, pt0:pt0 + 1],
                            rhs=ph[:, wuh * uh_w:(wuh + 1) * uh_w],
                            start=(pt0 == 0),
                            stop=(pt0 == nt - 1),
                        )
                wsum_pending = []

                # ---- normalization: o = o' / Z -----------------------
                esum = spool.tile([128, 1], F32, tag="esum", name="esum")
                nc.vector.reduce_sum(out=esum[:, :], in_=e_f32[:, :],
                                     axis=mybir.AxisListType.X)
                z_ps = zps.tile([1, 1], F32, tag="z_ps", name="z_ps")
                nc.tensor.matmul(out=z_ps[:, :], lhsT=ones_s[:, :],
                                 rhs=esum[:, :], start=True, stop=True)
                rz = spool.tile([1, 1], F32, tag="rz", name="rz")
                nc.vector.reciprocal(rz[:, :], z_ps[:, :])
                o_sb = scratch.tile([1, units], F32, tag="o_sb", name="o_sb")
                nc.vector.tensor_scalar_mul(o_sb[:, :], o_ps[:, :],
                                            rz[0:1, 0:1])
                nc.sync.dma_start(out=out[b:b + 1, :], in_=o_sb[:, :])

    nc.finalize()
    return nc


def _prep_inputs(sequences1, sequences2, W1_kernel, W1_bias, W2_kernel,
                 W2_bias, W_kernel, W_bias, context_vector):
    """Host-side folding + layout. Returns (per-core in_maps, has_bias)."""
    U = UNITS
    W = np.asarray(W_kernel, np.float32)
    M1 = np.asarray(W1_kernel, np.float32) @ W[:U]
    M2 = np.asarray(W2_kernel, np.float32) @ W[U:]
    beff = (np.asarray(W1_bias, np.float32) @ W[:U]
            + np.asarray(W2_bias, np.float32) @ W[U:]
            + np.asarray(W_bias, np.float32))
    has_bias = bool(np.any(beff != 0.0))

    M = np.concatenate([M1, M2], axis=0)                   # [KD, U]
    mw = np.ascontiguousarray(M.reshape(KD // 128, 128, U), np.float32)
    c = np.asarray(context_vector, np.float32).reshape(U)
    crep = np.ascontiguousarray(np.broadcast_to(c, (128, U)), np.float32)
    brep = np.ascontiguousarray(np.broadcast_to(beff, (128, U)), np.float32)

    x1 = np.asarray(sequences1, np.float32)
    x2 = np.asarray(sequences2, np.float32)
    in_maps = []
    for core in range(N_CORES):
        bs = slice(core * BPC, (core + 1) * BPC)
        xcat = np.concatenate([x1[bs], x2[bs]], axis=2)    # [BPC, T, KD]
        # -> [BPC, KD/128, 128, T]: xt[b, k, p, t] = xcat[b, t, 128k + p]
        xtc = np.ascontiguousarray(
            xcat.transpose(0, 2, 1).reshape(BPC, KD // 128, 128, T)
        )
        in_maps.append({"xt": xtc, "mw": mw, "crep": crep, "brep": brep})
    return in_maps, has_bias


def kernel(sequences1, sequences2, W1_kernel, W1_bias, W2_kernel, W2_bias,
           W_kernel, W_bias, context_vector):
    in_maps, has_bias = _prep_inputs(
        sequences1, sequences2, W1_kernel, W1_bias, W2_kernel, W2_bias,
        W_kernel, W_bias, context_vector)
    key = ("full", has_bias)
    if key not in _NC_CACHE:
        _NC_CACHE[key] = build_nc(has_bias=has_bias)
    nc = _NC_CACHE[key]
    res = run_bass_kernel_spmd(nc, in_maps, list(range(N_CORES)))
    return np.concatenate([r["out"] for r in res.results], axis=0)
